# revision 1
# baseline (speedup 1.0000x reference)
"""Distributed identity-copy kernel for nn_ExiLU_14147622273111.

The reference op is `where((x >= 1001) | (x <= 447113), x, 0)` — the
predicate is tautologically true, so the op is an identity copy of a
(4096, 32768) fp32 tensor. We shard row-blocks across 8 NeuronCores and
each core does a DRAM->DRAM DMA copy of its 64 MB shard (memory-bound;
read + write share the per-core HBM bandwidth).
"""

import numpy as np

import concourse.bass as bass
import concourse.mybir as mybir
from concourse.bass_utils import run_bass_kernel_spmd

N_CORES = 8
ROWS, COLS = 4096, 32768
SHARD_ROWS = ROWS // N_CORES  # 512 rows -> 64 MB fp32 per core

_nc_cache = None


def _build() -> bass.Bass:
    global _nc_cache
    if _nc_cache is not None:
        return _nc_cache
    nc = bass.Bass()
    x = nc.dram_tensor(
        "tensor", [SHARD_ROWS, COLS], mybir.dt.float32, kind="ExternalInput"
    )
    y = nc.dram_tensor(
        "out", [SHARD_ROWS, COLS], mybir.dt.float32, kind="ExternalOutput"
    )
    with nc.Block() as block, nc.semaphore("dma_sem") as dma_sem:

        @block.sync
        def _(sync):
            sync.dma_start(y[:, :], x[:, :]).then_inc(dma_sem, 16)
            sync.wait_ge(dma_sem, 16)

    _nc_cache = nc
    return nc


def _run(in_maps, **kwargs):
    nc = _build()
    return run_bass_kernel_spmd(nc, in_maps, core_ids=list(range(N_CORES)), **kwargs)


def kernel(tensor: np.ndarray) -> np.ndarray:
    tensor = np.ascontiguousarray(np.asarray(tensor, dtype=np.float32))
    assert tensor.shape == (ROWS, COLS), tensor.shape
    shards = np.split(tensor, N_CORES, axis=0)
    in_maps = [{"tensor": s} for s in shards]
    res = _run(in_maps)
    return np.concatenate([r["out"] for r in res.results], axis=0)


# revision 8
# speedup vs baseline: 1.1953x; 1.1953x over previous
"""Distributed identity-copy kernel for nn_ExiLU_14147622273111.

The reference op is `where((x >= 1001) | (x <= 447113), x, 0)` — the
predicate is tautologically true, so the op is an identity copy of a
(4096, 32768) fp32 tensor. We shard row-blocks across 8 NeuronCores and
each core copies its 64 MB shard DRAM->DRAM with the two HWDGE rings
(qSync + qScalar), which saturates all 16 SDMA engines at the per-core
HBM streaming limit (~330 GB/s payload, ~660 GB/s HBM traffic).

NeuronCore pairs (0,1),(2,3),(4,5),(6,7) share an HBM domain; two cores
copying concurrently halve each other's bandwidth. Under axon/PJRT we
therefore dispatch the eight 0.2 ms copies back-to-back (one jit per
core), so every core runs at the solo streaming rate; the ~1.5 ms of
serial device time is noise next to the host<->device transfers.
"""

import numpy as np

import concourse.bass as bass
import concourse.mybir as mybir
from concourse._compat import axon_active

N_CORES = 8
ROWS, COLS = 4096, 32768
SHARD_ROWS = ROWS // N_CORES  # 512 rows -> 64 MB fp32 per core

_nc_cache = {}
_exec_cache = {}


def _build() -> bass.Bass:
    if "nc" in _nc_cache:
        return _nc_cache["nc"]
    nc = bass.Bass()
    x = nc.dram_tensor(
        "tensor", [SHARD_ROWS, COLS], mybir.dt.float32, kind="ExternalInput"
    )
    y = nc.dram_tensor(
        "out", [SHARD_ROWS, COLS], mybir.dt.float32, kind="ExternalOutput"
    )
    # Split across the two HWDGE rings; no Block so no exit all-engine
    # barrier / DGE drains — completion is guaranteed by the DMA
    # semaphores (inc fires only after last-byte receipt in DRAM).
    h = SHARD_ROWS // 2
    with nc.semaphore("s_sem") as s_sem, nc.semaphore("a_sem") as a_sem:
        nc.sync.dma_start(y[:h, :], x[:h, :]).then_inc(s_sem, 16)
        nc.scalar.dma_start(y[h:, :], x[h:, :]).then_inc(a_sem, 16)
        nc.sync.wait_ge(s_sem, 16)
        nc.scalar.wait_ge(a_sem, 16)
    _nc_cache["nc"] = nc
    return nc


def _executables():
    """Build (fn, zeros_fn, sharding, devices), cached."""
    if "exe" in _exec_cache:
        return _exec_cache["exe"]

    import jax
    import jax.numpy as jnp
    from jax.sharding import Mesh, NamedSharding, PartitionSpec
    from jax.experimental.shard_map import shard_map
    from concourse.bass2jax import (
        _bass_exec_p,
        install_neuronx_cc_hook,
        partition_id_tensor,
    )

    install_neuronx_cc_hook()
    nc = _build()

    out_aval = jax.core.ShapedArray((SHARD_ROWS, COLS), np.float32)
    in_names = ["tensor", "out"]
    if nc.partition_id_tensor is not None:
        in_names.append(nc.partition_id_tensor.name)

    def _body(x, zero_out):
        operands = [x, zero_out]
        if nc.partition_id_tensor is not None:
            operands.append(partition_id_tensor())
        outs = _bass_exec_p.bind(
            *operands,
            out_avals=(out_aval,),
            in_names=tuple(in_names),
            out_names=("out",),
            lowering_input_output_aliases=(),
            sim_require_finite=True,
            sim_require_nnan=True,
            nc=nc,
        )
        return outs[0]

    devices = jax.devices()[:N_CORES]
    fn = jax.jit(_body, donate_argnums=(1,), keep_unused=True)
    # Donated output buffers created on-device (no 64 MB zero uploads,
    # which would contend with the copies for HBM bandwidth).
    zeros_fns = [
        jax.jit(
            lambda: jnp.zeros((SHARD_ROWS, COLS), np.float32),
            out_shardings=jax.sharding.SingleDeviceSharding(d),
        )
        for d in devices
    ]
    _exec_cache["exe"] = (fn, zeros_fns, devices)
    return _exec_cache["exe"]


def _run_axon(shards):
    """One jit execution per core, serialized: HBM-domain pair cores
    (0,1),(2,3),... halve each other's bandwidth when their copies
    overlap, so running the 0.2 ms copies back-to-back keeps every core
    at the solo streaming rate for ~1.5 ms total device time."""
    import jax

    fn, zeros_fns, devices = _executables()
    # Upload all shards first so host->HBM transfers don't overlap the
    # copies; read back only after every copy is done, for the same
    # reason.
    pieces = [jax.device_put(shards[i], devices[i]) for i in range(N_CORES)]
    zs = [zeros_fns[i]() for i in range(N_CORES)]
    for arr in (*pieces, *zs):
        jax.block_until_ready(arr)

    outs = []
    for i in range(N_CORES):
        out = fn(pieces[i], zs[i])
        jax.block_until_ready(out)
        outs.append(out)
    return np.concatenate([np.asarray(o) for o in outs], axis=0)


def _run_native(shards):
    from concourse.bass_utils import run_bass_kernel_spmd

    nc = _build()
    in_maps = [{"tensor": s} for s in shards]
    res = run_bass_kernel_spmd(nc, in_maps, core_ids=list(range(N_CORES)))
    return np.concatenate([r["out"] for r in res.results], axis=0)


def kernel(tensor: np.ndarray) -> np.ndarray:
    tensor = np.ascontiguousarray(np.asarray(tensor, dtype=np.float32))
    assert tensor.shape == (ROWS, COLS), tensor.shape
    shards = np.split(tensor, N_CORES, axis=0)
    if axon_active():
        return _run_axon(shards)
    return _run_native(shards)
